# revision 10
# baseline (speedup 1.0000x reference)
"""Trainium2 Bass kernel for nn_CAE_21242908246023 (moe_routing).

Computation (B=16384, D=5000, L=64):
  h_base = expr @ W_base.T                     [B, L]
  logits = h_base @ W_base                     [B, D]
  for ctx in (batch[card 24], cell[card 10]):
      shared = expr @ W_enc.T                  [B, L]
      h_f    = einsum('bl,bml->bm', shared, W_heads[src])
      dec    = einsum('bl,bml->bm', h_f,    W_heads[tgt])
      logits += (dec @ W_dec.T) * 0.0159

Strategy: data-parallel over B across 8 cores (2048 rows each), weights
replicated.  bf16 matmuls with fp32 PSUM accumulation, fp32 output.
Per core: expr^T tiles arrive via XBAR transpose-DMA; one fused encoder
matmul produces all three 64-dim latents in transposed layout; per-row
expert routing is computed all-experts then selected with one-hot masks
(mask multiply + tree reduction on DVE); the three decoder matmuls are
fused into a single PSUM accumulation against a stacked [192, 5000]
weight; output tiles stream out as contiguous 2.56MB DMAs.
"""

from contextlib import ExitStack

import ml_dtypes
import numpy as np

import concourse.bacc as bacc
import concourse.bass as bass
import concourse.mybir as mybir
import concourse.tile as tile
from concourse._compat import with_exitstack
from concourse.bass_utils import run_bass_kernel_spmd

BF16 = ml_dtypes.bfloat16

B, D, L = 16384, 5000, 64
CARD_B, CARD_C = 24, 10
DEC_SCALE = 0.0159
N_CORES = 8
R = B // N_CORES          # rows per core
DP = 5120                 # D padded to a multiple of 128
NK = DP // 128            # contraction chunks (40)
QW = 512                  # encoder quarter width (rows)
NOUT = 10                 # decoder output chunks of 500
OW = D // NOUT            # 500

f32 = mybir.dt.float32
bf16 = mybir.dt.bfloat16


def _ap3(ap, outer, inner):
    """[P, outer*inner] AP -> [P, outer, inner] view."""
    pdim, fdim = ap.ap[0], ap.ap[1]
    assert fdim[1] == outer * inner and fdim[0] == 1
    return bass.AP(ap.tensor, ap.offset, [pdim, [inner, outer], [1, inner]])


def _bcast(ap, n):
    """[P, E] AP -> [P, E, n] broadcast view (step-0 inner dim)."""
    return bass.AP(ap.tensor, ap.offset, ap.ap + [[0, n]])


@with_exitstack
def _kernel(ctx, tc, rows, io):
    nc = tc.nc
    nq = rows // QW
    nt = rows // 128

    x, wenc, wout1, wout2, wstb, wstc, msbs, msbt, mscs, msct, ident, y = io

    consts = ctx.enter_context(tc.tile_pool(name="consts", bufs=1))

    def load_const(ap, dtype):
        t = consts.tile(list(ap.shape), dtype, tag=ap.tensor.name)
        nc.sync.dma_start(t[:], ap)
        return t

    wenc_t = load_const(wenc, bf16)
    wout1_t = load_const(wout1, bf16)
    wout2_t = load_const(wout2, bf16)
    wstb_t = load_const(wstb, bf16)
    wstc_t = load_const(wstc, bf16)
    msbs_t = load_const(msbs, f32)
    msbt_t = load_const(msbt, f32)
    mscs_t = load_const(mscs, f32)
    msct_t = load_const(msct, f32)
    ident_t = load_const(ident, bf16)

    lat = ctx.enter_context(tc.tile_pool(name="lat", bufs=1))
    shB = lat.tile([64, rows], bf16, tag="shB")    # shared_batch^T
    shC = lat.tile([64, rows], bf16, tag="shC")    # shared_cell^T
    zT1 = lat.tile([128, rows], bf16, tag="zT1")   # [h_base^T ; dec_b^T]
    zT2 = lat.tile([64, rows], bf16, tag="zT2")    # dec_c^T

    xpool = ctx.enter_context(tc.tile_pool(name="xT", bufs=6))
    encps = ctx.enter_context(tc.tile_pool(name="encps", bufs=1, space="PSUM"))
    headps = ctx.enter_context(tc.tile_pool(name="headps", bufs=2, space="PSUM"))
    tpps = ctx.enter_context(tc.tile_pool(name="tpps", bufs=2, space="PSUM"))
    outps = ctx.enter_context(tc.tile_pool(name="outps", bufs=2, space="PSUM"))
    tmpp = ctx.enter_context(tc.tile_pool(name="tmpp", bufs=2))
    small = ctx.enter_context(tc.tile_pool(name="small", bufs=3))
    opool = ctx.enter_context(tc.tile_pool(name="osb", bufs=2))

    def select(ps_tiles, mask_t, moff, card, tag, out_ap):
        """Per-row expert selection: out[p, m] = sum_e mask[p, e] * ps[p, e*64+m].

        ps_tiles: list of ([128, w] psum AP, expert offset, n_experts).
        Writes the [128, 64] result to out_ap (bf16).
        """
        tmp = tmpp.tile([128, card * 64], bf16, tag=f"tmp{tag}")
        for ps, e0, ne in ps_tiles:
            nc.vector.tensor_mul(
                _ap3(tmp[:, e0 * 64:(e0 + ne) * 64], ne, 64),
                _ap3(ps, ne, 64),
                _bcast(mask_t[:, moff + e0:moff + e0 + ne], 64),
            )
        # tree-reduce over experts
        def halve(src, n):
            h = n // 2
            dst = small.tile([128, h * 64], bf16, tag=f"acc{tag}{h}")
            nc.vector.tensor_add(dst[:], src[:, :h * 64], src[:, h * 64:2 * h * 64])
            return dst, src[:, 2 * h * 64:] if n % 2 else None

        cur, n = tmp, card
        extras = []
        while n > 1:
            cur, rem = halve(cur, n)
            if rem is not None:
                extras.append(rem)
            n //= 2
        cur = cur[:]
        if extras:
            for ex in extras[:-1]:
                nxt = small.tile([128, 64], bf16, tag=f"hx{tag}")
                nc.vector.tensor_add(nxt[:], cur, ex)
                cur = nxt[:]
            nc.vector.tensor_add(out_ap, cur, extras[-1])
        else:
            nc.vector.tensor_copy(out_ap, cur)

    def head_chunks(lhsT_ap, wst_t, card):
        """All-experts matmuls: returns list of (psum AP, e0, ne)."""
        res = []
        total = card * 64
        c0 = 0
        while c0 < total:
            w = min(512, total - c0)
            ps = headps.tile([128, 512], f32, tag="hps")
            nc.tensor.matmul(ps[:, :w], lhsT_ap, wst_t[:, c0:c0 + w],
                             start=True, stop=True)
            res.append((ps[:, :w], c0 // 64, w // 64))
            c0 += w
        return res

    def transpose_pair(src_t, tag):
        """[128, 128] sbuf pair -> [128, 128] bf16 psum via one PE transpose."""
        tp = tpps.tile([128, 128], bf16, tag="tp")
        nc.tensor.transpose(tp[:], src_t[:], ident_t[:])
        return tp

    for q in range(nq):
        b0 = q * QW
        h1 = encps.tile([128, QW], f32, tag="h1")
        h2 = encps.tile([64, QW], f32, tag="h2")
        for k in range(NK):
            xt = xpool.tile([128, QW], bf16, tag="xt")
            nc.sync.dma_start(xt[:], x[b0:b0 + QW, k * 128:(k + 1) * 128],
                              transpose=True)
            nc.tensor.matmul(h1[:], wenc_t[:, k * 192:k * 192 + 128], xt[:],
                             start=(k == 0), stop=(k == NK - 1))
            nc.tensor.matmul(h2[:], wenc_t[:, k * 192 + 128:(k + 1) * 192], xt[:],
                             start=(k == 0), stop=(k == NK - 1))
        cp = mybir.ActivationFunctionType.Copy
        nc.scalar.activation(zT1[0:64, b0:b0 + QW], h1[0:64, :], cp)
        nc.scalar.activation(shB[:, b0:b0 + QW], h1[64:128, :], cp)
        nc.scalar.activation(shC[:, b0:b0 + QW], h2[:, :], cp)

        for ti in range(QW // 128):
            t = q * (QW // 128) + ti
            b = t * 128
            # --- stage 1 (src heads), batch in cols 0:64, cell in 64:128 ---
            hfp = small.tile([128, 128], bf16, tag="hfp")
            ps1 = head_chunks(shB[:, b:b + 128], wstb_t, CARD_B)
            select(ps1, msbs_t, t * CARD_B, CARD_B, "b1", hfp[:, 0:64])
            ps1 = head_chunks(shC[:, b:b + 128], wstc_t, CARD_C)
            select(ps1, mscs_t, t * CARD_C, CARD_C, "c1", hfp[:, 64:128])
            hfT = transpose_pair(hfp, "s1")
            hfTb = small.tile([64, 128], bf16, tag="hfTb")
            hfTc = small.tile([64, 128], bf16, tag="hfTc")
            nc.scalar.activation(hfTb[:], hfT[0:64, :], cp)
            nc.scalar.activation(hfTc[:], hfT[64:128, :], cp)
            # --- stage 2 (tgt heads) ---
            dcp = small.tile([128, 128], bf16, tag="dcp")
            ps2 = head_chunks(hfTb[:], wstb_t, CARD_B)
            select(ps2, msbt_t, t * CARD_B, CARD_B, "b2", dcp[:, 0:64])
            ps2 = head_chunks(hfTc[:], wstc_t, CARD_C)
            select(ps2, msct_t, t * CARD_C, CARD_C, "c2", dcp[:, 64:128])
            dcT = transpose_pair(dcp, "s2")
            nc.scalar.activation(zT1[64:128, b:b + 128], dcT[0:64, :], cp)
            nc.scalar.activation(zT2[:, b:b + 128], dcT[64:128, :], cp)
            # --- decoder ---
            osb = opool.tile([128, D], bf16, tag="osb")
            for n in range(NOUT):
                lp = outps.tile([128, OW], f32, tag="lp")
                nc.tensor.matmul(lp[:], zT1[:, b:b + 128],
                                 wout1_t[:, n * OW:(n + 1) * OW],
                                 start=True, stop=False)
                nc.tensor.matmul(lp[:], zT2[:, b:b + 128],
                                 wout2_t[:, n * OW:(n + 1) * OW],
                                 start=False, stop=True)
                if n % 3 == 2:
                    nc.vector.tensor_copy(osb[:, n * OW:(n + 1) * OW], lp[:])
                else:
                    nc.scalar.activation(osb[:, n * OW:(n + 1) * OW], lp[:], cp)
            nc.scalar.dma_start(y[b:b + 128, :], osb[:])


def _declare(nc, rows):
    def di(name, shape, dt):
        return nc.dram_tensor(name, shape, dt, kind="ExternalInput").ap()

    x = di("x", [rows, DP], bf16)
    wenc = di("wenc", [128, NK * 192], bf16)
    wout1 = di("wout1", [128, D], bf16)
    wout2 = di("wout2", [64, D], bf16)
    wstb = di("wstb", [64, CARD_B * 64], bf16)
    wstc = di("wstc", [64, CARD_C * 64], bf16)
    nt = rows // 128
    msbs = di("msbs", [128, nt * CARD_B], f32)
    msbt = di("msbt", [128, nt * CARD_B], f32)
    mscs = di("mscs", [128, nt * CARD_C], f32)
    msct = di("msct", [128, nt * CARD_C], f32)
    ident = di("ident", [128, 128], bf16)
    y = nc.dram_tensor("y", [rows, D], bf16, kind="ExternalOutput").ap()
    return [x, wenc, wout1, wout2, wstb, wstc, msbs, msbt, mscs, msct, ident, y]


_PROGRAMS = {}


def build_program(rows=R):
    if rows in _PROGRAMS:
        return _PROGRAMS[rows]
    nc = bacc.Bacc("TRN2", target_bir_lowering=False, debug=False,
                   num_devices=N_CORES if rows == R else 1)
    io = _declare(nc, rows)
    with tile.TileContext(nc) as tc:
        _kernel(tc, rows, io)
    nc.compile()
    _PROGRAMS[rows] = nc
    return nc


def prep_weights(W_base, W_enc_batch, W_dec_batch, W_heads_batch,
                 W_enc_cell, W_dec_cell, W_heads_cell):
    stackT = np.zeros((DP, 192), np.float32)
    stackT[:D, 0:64] = W_base.T
    stackT[:D, 64:128] = W_enc_batch.T
    stackT[:D, 128:192] = W_enc_cell.T
    wenc = np.ascontiguousarray(
        stackT.reshape(NK, 128, 192).transpose(1, 0, 2).reshape(128, NK * 192)
    ).astype(BF16)
    wout = np.concatenate(
        [W_base, DEC_SCALE * W_dec_batch.T, DEC_SCALE * W_dec_cell.T], axis=0
    ).astype(BF16)
    wstb = np.ascontiguousarray(
        W_heads_batch.transpose(2, 0, 1).reshape(64, CARD_B * 64)).astype(BF16)
    wstc = np.ascontiguousarray(
        W_heads_cell.transpose(2, 0, 1).reshape(64, CARD_C * 64)).astype(BF16)
    ident = np.eye(128, dtype=BF16)
    return {
        "wenc": wenc, "wout1": np.ascontiguousarray(wout[0:128]),
        "wout2": np.ascontiguousarray(wout[128:192]),
        "wstb": wstb, "wstc": wstc, "ident": ident,
    }


def prep_mask(idx, card):
    """[rows] int -> [128, (rows/128)*card] f32 one-hot in SBUF layout."""
    nt = idx.shape[0] // 128
    oh = (idx.reshape(nt, 128)[:, :, None] == np.arange(card)).astype(np.float32)
    return np.ascontiguousarray(oh.transpose(1, 0, 2).reshape(128, nt * card))


def prep_x(expr_rows):
    xp = np.zeros((expr_rows.shape[0], DP), BF16)
    xp[:, :D] = expr_rows.astype(BF16)
    return xp


def kernel(expr, src_batch, tgt_batch, src_cell, tgt_cell,
           W_base, W_enc_batch, W_dec_batch, W_heads_batch,
           W_enc_cell, W_dec_cell, W_heads_cell):
    nc = build_program(R)
    wmap = prep_weights(W_base, W_enc_batch, W_dec_batch, W_heads_batch,
                        W_enc_cell, W_dec_cell, W_heads_cell)
    in_maps = []
    for c in range(N_CORES):
        sl = slice(c * R, (c + 1) * R)
        in_maps.append({
            "x": prep_x(expr[sl]),
            "msbs": prep_mask(src_batch[sl], CARD_B),
            "msbt": prep_mask(tgt_batch[sl], CARD_B),
            "mscs": prep_mask(src_cell[sl], CARD_C),
            "msct": prep_mask(tgt_cell[sl], CARD_C),
            **wmap,
        })
    res = run_bass_kernel_spmd(nc, in_maps, core_ids=list(range(N_CORES)))
    global LAST_RESULT
    LAST_RESULT = res
    out = np.concatenate([res.results[c]["y"] for c in range(N_CORES)], axis=0)
    return np.asarray(out, dtype=np.float32)


LAST_RESULT = None
